# revision 42
# baseline (speedup 1.0000x reference)
"""Trainium2 Bass kernel for nn_CGNonLinearity3D (MAXL=2, CH=256, B=2048).

Algorithm (per core, batch-sharded x8):
  stage 1 (DVE): complex "diagonal CG" products via Gauss 3-mult, weighted
    accumulation into symmetry-reduced cat blocks (zero blocks dropped,
    (l2,l1) duplicates folded into host-merged weights).
  stage 2 (PE): per-(l,m) complex matmul as real f32r matmuls accumulating
    in PSUM; batch rows on PSUM partitions, merged weights as moving operand.
Output rows are [re(256) | im(256)]; host un-interleaves.
"""
import numpy as np
from math import factorial, sqrt
from contextlib import ExitStack

MAXL = 2
CH = 256
B = 2048
N_CORES = 8
B_LOC = B // N_CORES          # 256
BC = 128                      # batch chunk (2 chunks per core)
NCHT = CH // 128              # 2 channel partition-tiles

PROD_DT16 = "fp16"          # None (fp32) | "fp16" | "bf16" stage-1 product dtype

# ---------------------------------------------------------------- CG plan --


def _cg_coeff(j1, m1, j2, m2, j, m):
    if m1 + m2 != m or not (abs(j1 - j2) <= j <= j1 + j2):
        return 0.0
    f = factorial
    pref = sqrt((2 * j + 1) * f(j + j1 - j2) * f(j - j1 + j2) * f(j1 + j2 - j) / f(j1 + j2 + j + 1))
    pref *= sqrt(f(j + m) * f(j - m) * f(j1 - m1) * f(j1 + m1) * f(j2 - m2) * f(j2 + m2))
    kmin = max(0, j2 - j - m1, j1 - j + m2)
    kmax = min(j1 + j2 - j, j1 - m1, j2 + m2)
    s = 0.0
    for k in range(kmin, kmax + 1):
        s += (-1) ** k / (f(k) * f(j1 + j2 - j - k) * f(j1 - m1 - k)
                          * f(j2 + m2 - k) * f(j - j2 + m1 + k) * f(j - j1 - m2 + k))
    return pref * s


def _cg_matrix(l1, l2, l):
    M = np.zeros((2 * l1 + 1, 2 * l2 + 1, 2 * l + 1), dtype=np.float64)
    for i, m1 in enumerate(range(-l1, l1 + 1)):
        for j, m2 in enumerate(range(-l2, l2 + 1)):
            m = m1 + m2
            if -l <= m <= l:
                M[i, j, m + l] = _cg_coeff(l1, m1, l2, m2, l, m)
    return M


PAIRS = {l: [(l1, l2) for l1 in range(MAXL + 1) for l2 in range(MAXL + 1)
             if abs(l1 - l2) <= l <= l1 + l2] for l in range(MAXL + 1)}


def build_plan():
    unique_pairs, wmerge = {}, {}
    for l in range(MAXL + 1):
        ups, wm = [], []
        for pi, (l1, l2) in enumerate(PAIRS[l]):
            if l1 > l2:
                continue
            sign = (-1) ** (l1 + l2 - l)
            if l1 == l2 and sign == -1:
                continue
            merge = [(pi, 1.0)]
            if l1 != l2 and (l2, l1) in PAIRS[l]:
                merge.append((PAIRS[l].index((l2, l1)), float(sign)))
            ups.append((l1, l2))
            wm.append(merge)
        unique_pairs[l] = ups
        wmerge[l] = wm

    entries = {}   # ((l1,l2), i, j) -> [(l, u_idx, m, coeff)]
    for l in range(MAXL + 1):
        for u_idx, (l1, l2) in enumerate(unique_pairs[l]):
            M = _cg_matrix(l1, l2, l)
            for i in range(2 * l1 + 1):
                for j in range(2 * l2 + 1):
                    if l1 == l2 and j < i:
                        continue
                    for mm in range(2 * l + 1):
                        c = M[i, j, mm] + (M[j, i, mm] if (l1 == l2 and j != i) else 0.0)
                        if abs(c) < 1e-12:
                            continue
                        entries.setdefault(((l1, l2), i, j), []).append((l, u_idx, mm, float(c)))
    return unique_pairs, wmerge, entries


UNIQUE_PAIRS, WMERGE, ENTRIES = build_plan()
GROUP_ORDER = [(1, 1), (2, 2), (1, 2), (0, 2), (0, 0), (0, 1)]
L_READY = {2: 3, 0: 4, 1: 5}
N_U = {l: len(UNIQUE_PAIRS[l]) for l in range(MAXL + 1)}
KT = {l: 2 * N_U[l] * NCHT for l in range(MAXL + 1)}   # k'-tiles per l


def _plan_products():
    prods = {}
    for g in GROUP_ORDER:
        used = sorted((i, j) for (gg, i, j) in ENTRIES if gg == g)
        by_i = {}
        for i, j in used:
            by_i.setdefault(i, []).append(j)
        runs = []
        for i, js in sorted(by_i.items()):
            js = sorted(js)
            lo = prev = js[0]
            for j in js[1:]:
                if j == prev + 1:
                    prev = j
                else:
                    runs.append((i, lo, prev))
                    lo = prev = j
            runs.append((i, lo, prev))
        prods[g] = runs
    return prods


PROD_RUNS = _plan_products()


# ------------------------------------------------------------- host prep --


def _host_pack_inputs(x0, x1, x2, w0, w1, w2):
    xs = {0: np.asarray(x0), 1: np.asarray(x1), 2: np.asarray(x2)}
    ws = {0: np.asarray(w0), 1: np.asarray(w1), 2: np.asarray(w2)}
    xdt = np.dtype("float32")
    if PROD_DT16 == "fp16":
        xdt = np.dtype("float16")
    elif PROD_DT16 == "bf16":
        import ml_dtypes
        xdt = np.dtype(ml_dtypes.bfloat16)

    wtiles = {}
    for l in range(MAXL + 1):
        wc = ws[l][..., 0].astype(np.float32) + 1j * ws[l][..., 1].astype(np.float32)
        pieces = []
        for merge in WMERGE[l]:
            acc = np.zeros((CH, CH), dtype=np.complex64)
            for pi, sgn in merge:
                acc += sgn * wc[pi * CH:(pi + 1) * CH, :]
            pieces.append(acc)
        wt = np.concatenate(pieces, axis=0)          # [n_u*CH, CH] complex
        re, im = wt.real.astype(np.float32), wt.imag.astype(np.float32)
        top = np.concatenate([re, im], axis=1)       # plane 0 rows
        bot = np.concatenate([-im, re], axis=1)      # plane 1 rows
        Wp = np.concatenate([top, bot], axis=0)      # [2*n_u*CH, 512]
        Wp = Wp.reshape(KT[l], 128, 512)
        if PROD_DT16 == "fp16":
            Wp = Wp.astype(np.float16)
        wtiles[l] = np.ascontiguousarray(Wp)

    in_maps = []
    for c in range(N_CORES):
        m = {}
        for l in range(MAXL + 1):
            xl = xs[l][c * B_LOC:(c + 1) * B_LOC]     # [256, 2l+1, 256, 2]
            # -> [chunk, ri, chA(128 partitions), m, chB, bc]
            xt = xl.transpose(2, 3, 1, 0).reshape(2, 128, 2, 2 * l + 1, 2, BC)
            xt = xt.transpose(4, 2, 1, 3, 0, 5)
            m[f"x{l}"] = np.ascontiguousarray(xt.astype(np.float32)).astype(xdt)
            m[f"w{l}"] = wtiles[l]
        in_maps.append(m)
    return in_maps


# ---------------------------------------------------------- device kernel --


def _build_bass():
    import concourse.bass as bass
    import concourse.bacc as bacc
    import concourse.tile as tile
    from concourse import mybir

    f32 = mybir.dt.float32
    f32r = mybir.dt.float32r
    pdt = {None: f32, "fp16": mybir.dt.float16, "bf16": mybir.dt.bfloat16}[PROD_DT16]
    wdt = mybir.dt.float16 if PROD_DT16 == "fp16" else f32r
    mult = mybir.AluOpType.mult
    add = mybir.AluOpType.add
    sub = mybir.AluOpType.subtract

    nc = bacc.Bacc("TRN2", target_bir_lowering=False, debug=False)

    xd, wd, od = {}, {}, {}
    for l in range(MAXL + 1):
        nm = 2 * l + 1
        xd[l] = nc.dram_tensor(f"x{l}", [2, 2, 128, nm * 2 * BC], pdt, kind="ExternalInput").ap()
        wd[l] = nc.dram_tensor(f"w{l}", [KT[l], 128, 512], wdt, kind="ExternalInput").ap()
        od[l] = nc.dram_tensor(f"out{l}", [B_LOC, nm, 512], f32, kind="ExternalOutput").ap()

    n_per_block = {}
    for tgts in ENTRIES.values():
        for (l, u, mm, cf) in tgts:
            n_per_block[(l, u, mm)] = n_per_block.get((l, u, mm), 0) + 1

    with tile.TileContext(nc) as tc:
        ctx = ExitStack()
        with ctx:
            xpool = ctx.enter_context(tc.tile_pool(name="xp", bufs=2))
            stpool = ctx.enter_context(tc.tile_pool(name="st", bufs=1))
            kpool = ctx.enter_context(tc.tile_pool(name="kp", bufs=2))
            ppool = ctx.enter_context(tc.tile_pool(name="pp", bufs=2))
            bpool = ctx.enter_context(tc.tile_pool(name="bp", bufs=2))
            wpool = ctx.enter_context(tc.tile_pool(name="wp", bufs=1))
            opool = ctx.enter_context(tc.tile_pool(name="op", bufs=4))
            pspool = ctx.enter_context(tc.tile_pool(name="ps", bufs=8, space="PSUM"))

            wt = {}

            blocks = {}
            remaining = {}
            CB = 2 * BC   # (chB, bc) super-index: 128 high channels x 128 batch

            def emit_macs(g, i, j, P):
                """P: [128,512] (re256|im256, inner (chB,bc)) slice for (g,i,j)."""
                for (l, u, mm, cf) in ENTRIES[(g, i, j)]:
                    bkc = (l, u, mm)
                    if bkc not in blocks or remaining[bkc] == n_per_block[bkc]:
                        if bkc not in blocks:
                            blocks[bkc] = bpool.tile([128, 512], wdt, tag=f"b{l}_{u}_{mm}", name=f"b{l}_{u}_{mm}")
                            remaining[bkc] = n_per_block[bkc]
                        bt = blocks[bkc]
                        nc.vector.tensor_scalar(out=bt[:], in0=P, scalar1=cf, scalar2=None, op0=mult)
                    else:
                        bt = blocks[bkc]
                        nc.vector.scalar_tensor_tensor(out=bt[:], in0=P, scalar=cf, in1=bt[:], op0=mult, op1=add)
                    remaining[bkc] -= 1
                    if remaining[bkc] == 0:
                        remaining[bkc] = n_per_block[bkc]  # reset for next chunk

            def emit_matmuls(l, chunk):
                nm = 2 * l + 1
                for mm in range(nm):
                    ps = pspool.tile([128, 512], f32, tag="ps", name="ps")
                    T = KT[l]
                    for t in range(T):
                        plane = t // (N_U[l] * 2)
                        rem = t % (N_U[l] * 2)
                        u = rem // 2
                        chb = rem % 2
                        bt = blocks[(l, u, mm)]
                        off = plane * 256 + chb * 128
                        nc.tensor.matmul(ps[:], bt[:, off:off + 128], wt[(l, t)][:],
                                         start=(t == 0), stop=(t == T - 1))
                    ot = opool.tile([128, 512], f32, tag="ot", name="ot")
                    nc.scalar.copy(ot[:], ps[:])
                    nc.sync.dma_start(od[l][chunk * BC:(chunk + 1) * BC, mm], ot[:])

            # ---- per chunk: load x chunk, s/tm, products -> MACs -> matmuls ----
            for chunk in range(2):
                xt, st, tmt = {}, {}, {}
                for l in range(MAXL + 1):
                    nm = 2 * l + 1
                    for ri in range(2):
                        t = xpool.tile([128, nm * CB], pdt, tag=f"x{l}_{ri}", name=f"x{l}_{ri}")
                        nc.sync.dma_start(t[:], xd[l][chunk, ri])
                        xt[(l, ri)] = t
                    s = stpool.tile([128, nm * CB], pdt, tag=f"s{l}", name=f"s{l}")
                    nc.vector.tensor_tensor(s[:], xt[(l, 0)][:], xt[(l, 1)][:], add)
                    st[l] = s
                    tm = stpool.tile([128, nm * CB], pdt, tag=f"tm{l}", name=f"tm{l}")
                    nc.vector.tensor_tensor(tm[:], xt[(l, 1)][:], xt[(l, 0)][:], sub)
                    tmt[l] = tm

                if chunk == 0:
                    # preload W tiles now (after x DMAs) so DVE starts early
                    for l in (2, 0, 1):
                        for t in range(KT[l]):
                            w = wpool.tile([128, 512], wdt, tag=f"w{l}_{t}", name=f"w{l}_{t}")
                            nc.sync.dma_start(w[:], wd[l][t])
                            wt[(l, t)] = w

                for gi, g in enumerate(GROUP_ORDER):
                    l1, l2 = g
                    for (i, vlo, vhi) in PROD_RUNS[g]:
                        nv = vhi - vlo + 1

                        s1 = st[l1][:, i * CB:(i + 1) * CB]
                        xre1 = xt[(l1, 0)][:, i * CB:(i + 1) * CB]
                        xim1 = xt[(l1, 1)][:, i * CB:(i + 1) * CB]

                        P = ppool.tile([128, nv * 512], pdt, tag="pg", name="pg")
                        Pv = P[:].rearrange("p (v two cb) -> p v two cb", v=nv, two=2)
                        Pre = Pv[:, :, 0, :]
                        Pim = Pv[:, :, 1, :]

                        def bc(ap):
                            return ap.unsqueeze(1).broadcast_to((128, nv, CB))

                        def vsl(tile_):
                            # 3D view [128, nv, CB] over the v-run
                            nmv = tile_.shape[1] // CB
                            t3 = tile_[:].rearrange("p (v cb) -> p v cb", v=nmv)
                            return t3[:, vlo:vhi + 1, :]

                        xre2v = vsl(xt[(l2, 0)])
                        tm2v = vsl(tmt[l2])
                        s2v = vsl(st[l2])

                        # pass1: P.re = K1 = xre_v * s_u
                        nc.vector.tensor_tensor(Pre, xre2v, bc(s1), mult)
                        # pass2: P.im = K2 = tm_v * xre_u
                        nc.vector.tensor_tensor(Pim, tm2v, bc(xre1), mult)
                        # pass3: P.im += P.re  (k1+k2)
                        nc.vector.tensor_tensor(Pim, Pim, Pre, add)
                        # pass4: K3 = s_v * xim_u
                        K3 = kpool.tile([128, nv * CB], pdt, tag="k3", name="k3")
                        K3v = K3[:].rearrange("p (v cb) -> p v cb", v=nv)
                        nc.vector.tensor_tensor(K3v, s2v, bc(xim1), mult)
                        # pass5: P.re -= K3
                        nc.vector.tensor_tensor(Pre, Pre, K3v, sub)

                        for j in range(vlo, vhi + 1):
                            if ((l1, l2), i, j) not in ENTRIES:
                                continue
                            vloc = j - vlo
                            emit_macs(g, i, j, P[:, vloc * 512:(vloc + 1) * 512])

                    for l in range(MAXL + 1):
                        if L_READY[l] == gi:
                            emit_matmuls(l, chunk)

    nc.compile()
    return nc


_NC_CACHE = {}


def _get_nc():
    key = ("nc", PROD_DT16)
    if key not in _NC_CACHE:
        _NC_CACHE[key] = _build_bass()
    return _NC_CACHE[key]


def kernel(x0, x1, x2, w0, w1, w2):
    from concourse import bass_utils

    in_maps = _host_pack_inputs(x0, x1, x2, w0, w1, w2)
    nc = _get_nc()
    try:
        res = bass_utils.run_bass_kernel_spmd(nc, in_maps, core_ids=list(range(N_CORES)))
    except Exception:
        # transient device faults (NRT unrecoverable) clear on retry
        import time
        time.sleep(2.0)
        res = bass_utils.run_bass_kernel_spmd(nc, in_maps, core_ids=list(range(N_CORES)))
    outs = []
    for l in range(MAXL + 1):
        parts = [res.results[c][f"out{l}"].reshape(B_LOC, 2 * l + 1, 2, CH)
                 .transpose(0, 1, 3, 2)
                 for c in range(N_CORES)]
        outs.append(np.ascontiguousarray(np.concatenate(parts, axis=0)))
    return tuple(outs)


# revision 43
# speedup vs baseline: 1.0098x; 1.0098x over previous
"""Trainium2 Bass kernel for nn_CGNonLinearity3D (MAXL=2, CH=256, B=2048).

Algorithm (per core, batch-sharded x8):
  stage 1 (DVE): complex "diagonal CG" products via Gauss 3-mult, weighted
    accumulation into symmetry-reduced cat blocks (zero blocks dropped,
    (l2,l1) duplicates folded into host-merged weights).
  stage 2 (PE): per-(l,m) complex matmul as real f32r matmuls accumulating
    in PSUM; batch rows on PSUM partitions, merged weights as moving operand.
Output rows are [re(256) | im(256)]; host un-interleaves.
"""
import numpy as np
from math import factorial, sqrt
from contextlib import ExitStack

MAXL = 2
CH = 256
B = 2048
N_CORES = 8
B_LOC = B // N_CORES          # 256
BC = 128                      # batch chunk (2 chunks per core)
NCHT = CH // 128              # 2 channel partition-tiles

PROD_DT16 = "fp16"          # None (fp32) | "fp16" | "bf16" stage-1 product dtype

# ---------------------------------------------------------------- CG plan --


def _cg_coeff(j1, m1, j2, m2, j, m):
    if m1 + m2 != m or not (abs(j1 - j2) <= j <= j1 + j2):
        return 0.0
    f = factorial
    pref = sqrt((2 * j + 1) * f(j + j1 - j2) * f(j - j1 + j2) * f(j1 + j2 - j) / f(j1 + j2 + j + 1))
    pref *= sqrt(f(j + m) * f(j - m) * f(j1 - m1) * f(j1 + m1) * f(j2 - m2) * f(j2 + m2))
    kmin = max(0, j2 - j - m1, j1 - j + m2)
    kmax = min(j1 + j2 - j, j1 - m1, j2 + m2)
    s = 0.0
    for k in range(kmin, kmax + 1):
        s += (-1) ** k / (f(k) * f(j1 + j2 - j - k) * f(j1 - m1 - k)
                          * f(j2 + m2 - k) * f(j - j2 + m1 + k) * f(j - j1 - m2 + k))
    return pref * s


def _cg_matrix(l1, l2, l):
    M = np.zeros((2 * l1 + 1, 2 * l2 + 1, 2 * l + 1), dtype=np.float64)
    for i, m1 in enumerate(range(-l1, l1 + 1)):
        for j, m2 in enumerate(range(-l2, l2 + 1)):
            m = m1 + m2
            if -l <= m <= l:
                M[i, j, m + l] = _cg_coeff(l1, m1, l2, m2, l, m)
    return M


PAIRS = {l: [(l1, l2) for l1 in range(MAXL + 1) for l2 in range(MAXL + 1)
             if abs(l1 - l2) <= l <= l1 + l2] for l in range(MAXL + 1)}


def build_plan():
    unique_pairs, wmerge = {}, {}
    for l in range(MAXL + 1):
        ups, wm = [], []
        for pi, (l1, l2) in enumerate(PAIRS[l]):
            if l1 > l2:
                continue
            sign = (-1) ** (l1 + l2 - l)
            if l1 == l2 and sign == -1:
                continue
            merge = [(pi, 1.0)]
            if l1 != l2 and (l2, l1) in PAIRS[l]:
                merge.append((PAIRS[l].index((l2, l1)), float(sign)))
            ups.append((l1, l2))
            wm.append(merge)
        unique_pairs[l] = ups
        wmerge[l] = wm

    entries = {}   # ((l1,l2), i, j) -> [(l, u_idx, m, coeff)]
    for l in range(MAXL + 1):
        for u_idx, (l1, l2) in enumerate(unique_pairs[l]):
            M = _cg_matrix(l1, l2, l)
            for i in range(2 * l1 + 1):
                for j in range(2 * l2 + 1):
                    if l1 == l2 and j < i:
                        continue
                    for mm in range(2 * l + 1):
                        c = M[i, j, mm] + (M[j, i, mm] if (l1 == l2 and j != i) else 0.0)
                        if abs(c) < 1e-12:
                            continue
                        entries.setdefault(((l1, l2), i, j), []).append((l, u_idx, mm, float(c)))
    return unique_pairs, wmerge, entries


UNIQUE_PAIRS, WMERGE, ENTRIES = build_plan()
GROUP_ORDER = [(1, 1), (2, 2), (1, 2), (0, 2), (0, 0), (0, 1)]
L_READY = {2: 3, 0: 4, 1: 5}
N_U = {l: len(UNIQUE_PAIRS[l]) for l in range(MAXL + 1)}
KT = {l: 2 * N_U[l] * NCHT for l in range(MAXL + 1)}   # k'-tiles per l


def _plan_products():
    prods = {}
    for g in GROUP_ORDER:
        used = sorted((i, j) for (gg, i, j) in ENTRIES if gg == g)
        by_i = {}
        for i, j in used:
            by_i.setdefault(i, []).append(j)
        runs = []
        for i, js in sorted(by_i.items()):
            js = sorted(js)
            lo = prev = js[0]
            for j in js[1:]:
                if j == prev + 1:
                    prev = j
                else:
                    runs.append((i, lo, prev))
                    lo = prev = j
            runs.append((i, lo, prev))
        prods[g] = runs
    return prods


PROD_RUNS = _plan_products()


# ------------------------------------------------------------- host prep --


def _host_pack_inputs(x0, x1, x2, w0, w1, w2):
    xs = {0: np.asarray(x0), 1: np.asarray(x1), 2: np.asarray(x2)}
    ws = {0: np.asarray(w0), 1: np.asarray(w1), 2: np.asarray(w2)}
    xdt = np.dtype("float32")
    if PROD_DT16 == "fp16":
        xdt = np.dtype("float16")
    elif PROD_DT16 == "bf16":
        import ml_dtypes
        xdt = np.dtype(ml_dtypes.bfloat16)

    wtiles = {}
    for l in range(MAXL + 1):
        wc = ws[l][..., 0].astype(np.float32) + 1j * ws[l][..., 1].astype(np.float32)
        pieces = []
        for merge in WMERGE[l]:
            acc = np.zeros((CH, CH), dtype=np.complex64)
            for pi, sgn in merge:
                acc += sgn * wc[pi * CH:(pi + 1) * CH, :]
            pieces.append(acc)
        wt = np.concatenate(pieces, axis=0)          # [n_u*CH, CH] complex
        re, im = wt.real.astype(np.float32), wt.imag.astype(np.float32)
        top = np.concatenate([re, im], axis=1)       # plane 0 rows
        bot = np.concatenate([-im, re], axis=1)      # plane 1 rows
        Wp = np.concatenate([top, bot], axis=0)      # [2*n_u*CH, 512]
        Wp = Wp.reshape(KT[l], 128, 512)
        if PROD_DT16 == "fp16":
            Wp = Wp.astype(np.float16)
        wtiles[l] = np.ascontiguousarray(Wp)

    in_maps = []
    for c in range(N_CORES):
        m = {}
        for l in range(MAXL + 1):
            xl = xs[l][c * B_LOC:(c + 1) * B_LOC]     # [256, 2l+1, 256, 2]
            # -> [chunk, ri, chA(128 partitions), m, chB, bc]
            xt = xl.transpose(2, 3, 1, 0).reshape(2, 128, 2, 2 * l + 1, 2, BC)
            xt = xt.transpose(4, 2, 1, 3, 0, 5)
            m[f"x{l}"] = np.ascontiguousarray(xt.astype(np.float32)).astype(xdt)
            m[f"w{l}"] = wtiles[l]
        in_maps.append(m)
    return in_maps


# ---------------------------------------------------------- device kernel --


def _build_bass():
    import concourse.bass as bass
    import concourse.bacc as bacc
    import concourse.tile as tile
    from concourse import mybir

    f32 = mybir.dt.float32
    f32r = mybir.dt.float32r
    pdt = {None: f32, "fp16": mybir.dt.float16, "bf16": mybir.dt.bfloat16}[PROD_DT16]
    wdt = mybir.dt.float16 if PROD_DT16 == "fp16" else f32r
    mult = mybir.AluOpType.mult
    add = mybir.AluOpType.add
    sub = mybir.AluOpType.subtract

    nc = bacc.Bacc("TRN2", target_bir_lowering=False, debug=False)

    xd, wd, od = {}, {}, {}
    for l in range(MAXL + 1):
        nm = 2 * l + 1
        xd[l] = nc.dram_tensor(f"x{l}", [2, 2, 128, nm * 2 * BC], pdt, kind="ExternalInput").ap()
        wd[l] = nc.dram_tensor(f"w{l}", [KT[l], 128, 512], wdt, kind="ExternalInput").ap()
        od[l] = nc.dram_tensor(f"out{l}", [B_LOC, nm, 512], f32, kind="ExternalOutput").ap()

    n_per_block = {}
    for tgts in ENTRIES.values():
        for (l, u, mm, cf) in tgts:
            n_per_block[(l, u, mm)] = n_per_block.get((l, u, mm), 0) + 1

    with tile.TileContext(nc) as tc:
        ctx = ExitStack()
        with ctx:
            xpool = ctx.enter_context(tc.tile_pool(name="xp", bufs=2))
            stpool = ctx.enter_context(tc.tile_pool(name="st", bufs=1))
            kpool = ctx.enter_context(tc.tile_pool(name="kp", bufs=2))
            ppool = ctx.enter_context(tc.tile_pool(name="pp", bufs=2))
            bpool = ctx.enter_context(tc.tile_pool(name="bp", bufs=2))
            wpool = ctx.enter_context(tc.tile_pool(name="wp", bufs=1))
            opool = ctx.enter_context(tc.tile_pool(name="op", bufs=3))
            pspool = ctx.enter_context(tc.tile_pool(name="ps", bufs=6, space="PSUM"))

            wt = {}

            blocks = {}
            remaining = {}
            CB = 2 * BC   # (chB, bc) super-index: 128 high channels x 128 batch

            def emit_macs(g, i, j, P):
                """P: [128,512] (re256|im256, inner (chB,bc)) slice for (g,i,j)."""
                for (l, u, mm, cf) in ENTRIES[(g, i, j)]:
                    bkc = (l, u, mm)
                    if bkc not in blocks or remaining[bkc] == n_per_block[bkc]:
                        if bkc not in blocks:
                            blocks[bkc] = bpool.tile([128, 512], wdt, tag=f"b{l}_{u}_{mm}", name=f"b{l}_{u}_{mm}")
                            remaining[bkc] = n_per_block[bkc]
                        bt = blocks[bkc]
                        nc.vector.tensor_scalar(out=bt[:], in0=P, scalar1=cf, scalar2=None, op0=mult)
                    else:
                        bt = blocks[bkc]
                        nc.vector.scalar_tensor_tensor(out=bt[:], in0=P, scalar=cf, in1=bt[:], op0=mult, op1=add)
                    remaining[bkc] -= 1
                    if remaining[bkc] == 0:
                        remaining[bkc] = n_per_block[bkc]  # reset for next chunk

            def emit_matmuls(l, chunk):
                nm = 2 * l + 1
                for mm in range(nm):
                    ps = pspool.tile([128, 512], f32, tag="ps", name="ps")
                    T = KT[l]
                    for t in range(T):
                        plane = t // (N_U[l] * 2)
                        rem = t % (N_U[l] * 2)
                        u = rem // 2
                        chb = rem % 2
                        bt = blocks[(l, u, mm)]
                        off = plane * 256 + chb * 128
                        nc.tensor.matmul(ps[:], bt[:, off:off + 128], wt[(l, t)][:],
                                         start=(t == 0), stop=(t == T - 1))
                    ot = opool.tile([128, 512], f32, tag="ot", name="ot")
                    nc.scalar.copy(ot[:], ps[:])
                    nc.sync.dma_start(od[l][chunk * BC:(chunk + 1) * BC, mm], ot[:])

            # ---- per chunk: load x chunk, s/tm, products -> MACs -> matmuls ----
            for chunk in range(2):
                xt, st, tmt = {}, {}, {}
                for l in range(MAXL + 1):
                    nm = 2 * l + 1
                    for ri in range(2):
                        t = xpool.tile([128, nm * CB], pdt, tag=f"x{l}_{ri}", name=f"x{l}_{ri}")
                        nc.sync.dma_start(t[:], xd[l][chunk, ri])
                        xt[(l, ri)] = t
                    s = stpool.tile([128, nm * CB], pdt, tag=f"s{l}", name=f"s{l}")
                    nc.vector.tensor_tensor(s[:], xt[(l, 0)][:], xt[(l, 1)][:], add)
                    st[l] = s
                    tm = stpool.tile([128, nm * CB], pdt, tag=f"tm{l}", name=f"tm{l}")
                    nc.vector.tensor_tensor(tm[:], xt[(l, 1)][:], xt[(l, 0)][:], sub)
                    tmt[l] = tm

                if chunk == 0:
                    # preload W tiles now (after x DMAs) so DVE starts early
                    for l in (2, 0, 1):
                        for t in range(KT[l]):
                            w = wpool.tile([128, 512], wdt, tag=f"w{l}_{t}", name=f"w{l}_{t}")
                            nc.sync.dma_start(w[:], wd[l][t])
                            wt[(l, t)] = w

                for gi, g in enumerate(GROUP_ORDER):
                    l1, l2 = g
                    for (i, vlo, vhi) in PROD_RUNS[g]:
                        nv = vhi - vlo + 1

                        s1 = st[l1][:, i * CB:(i + 1) * CB]
                        xre1 = xt[(l1, 0)][:, i * CB:(i + 1) * CB]
                        xim1 = xt[(l1, 1)][:, i * CB:(i + 1) * CB]

                        P = ppool.tile([128, nv * 512], pdt, tag="pg", name="pg")
                        Pv = P[:].rearrange("p (v two cb) -> p v two cb", v=nv, two=2)
                        Pre = Pv[:, :, 0, :]
                        Pim = Pv[:, :, 1, :]

                        def bc(ap):
                            return ap.unsqueeze(1).broadcast_to((128, nv, CB))

                        def vsl(tile_):
                            # 3D view [128, nv, CB] over the v-run
                            nmv = tile_.shape[1] // CB
                            t3 = tile_[:].rearrange("p (v cb) -> p v cb", v=nmv)
                            return t3[:, vlo:vhi + 1, :]

                        xre2v = vsl(xt[(l2, 0)])
                        tm2v = vsl(tmt[l2])
                        s2v = vsl(st[l2])

                        # pass1: P.re = K1 = xre_v * s_u
                        nc.vector.tensor_tensor(Pre, xre2v, bc(s1), mult)
                        # pass2: P.im = K2 = tm_v * xre_u
                        nc.vector.tensor_tensor(Pim, tm2v, bc(xre1), mult)
                        # pass3: P.im += P.re  (k1+k2)
                        nc.vector.tensor_tensor(Pim, Pim, Pre, add)
                        # pass4: K3 = s_v * xim_u
                        K3 = kpool.tile([128, nv * CB], pdt, tag="k3", name="k3")
                        K3v = K3[:].rearrange("p (v cb) -> p v cb", v=nv)
                        nc.vector.tensor_tensor(K3v, s2v, bc(xim1), mult)
                        # pass5: P.re -= K3
                        nc.vector.tensor_tensor(Pre, Pre, K3v, sub)

                        for j in range(vlo, vhi + 1):
                            if ((l1, l2), i, j) not in ENTRIES:
                                continue
                            vloc = j - vlo
                            emit_macs(g, i, j, P[:, vloc * 512:(vloc + 1) * 512])

                    for l in range(MAXL + 1):
                        if L_READY[l] == gi:
                            emit_matmuls(l, chunk)

    nc.compile()
    return nc


_NC_CACHE = {}


def _get_nc():
    key = ("nc", PROD_DT16)
    if key not in _NC_CACHE:
        _NC_CACHE[key] = _build_bass()
    return _NC_CACHE[key]


def kernel(x0, x1, x2, w0, w1, w2):
    from concourse import bass_utils

    in_maps = _host_pack_inputs(x0, x1, x2, w0, w1, w2)
    nc = _get_nc()
    try:
        res = bass_utils.run_bass_kernel_spmd(nc, in_maps, core_ids=list(range(N_CORES)))
    except Exception:
        # transient device faults (NRT unrecoverable) clear on retry
        import time
        time.sleep(2.0)
        res = bass_utils.run_bass_kernel_spmd(nc, in_maps, core_ids=list(range(N_CORES)))
    outs = []
    for l in range(MAXL + 1):
        parts = [res.results[c][f"out{l}"].reshape(B_LOC, 2 * l + 1, 2, CH)
                 .transpose(0, 1, 3, 2)
                 for c in range(N_CORES)]
        outs.append(np.ascontiguousarray(np.concatenate(parts, axis=0)))
    return tuple(outs)


# revision 45
# speedup vs baseline: 1.0129x; 1.0031x over previous
"""Trainium2 Bass kernel for nn_CGNonLinearity3D (MAXL=2, CH=256, B=2048).

Algorithm (per core, batch-sharded x8):
  stage 1 (DVE): complex "diagonal CG" products via Gauss 3-mult, weighted
    accumulation into symmetry-reduced cat blocks (zero blocks dropped,
    (l2,l1) duplicates folded into host-merged weights).
  stage 2 (PE): per-(l,m) complex matmul as real f32r matmuls accumulating
    in PSUM; batch rows on PSUM partitions, merged weights as moving operand.
Output rows are [re(256) | im(256)]; host un-interleaves.
"""
import numpy as np
from math import factorial, sqrt
from contextlib import ExitStack

MAXL = 2
CH = 256
B = 2048
N_CORES = 8
B_LOC = B // N_CORES          # 256
BC = 128                      # batch chunk (2 chunks per core)
NCHT = CH // 128              # 2 channel partition-tiles

PROD_DT16 = "fp16"          # None (fp32) | "fp16" | "bf16" stage-1 product dtype

# ---------------------------------------------------------------- CG plan --


def _cg_coeff(j1, m1, j2, m2, j, m):
    if m1 + m2 != m or not (abs(j1 - j2) <= j <= j1 + j2):
        return 0.0
    f = factorial
    pref = sqrt((2 * j + 1) * f(j + j1 - j2) * f(j - j1 + j2) * f(j1 + j2 - j) / f(j1 + j2 + j + 1))
    pref *= sqrt(f(j + m) * f(j - m) * f(j1 - m1) * f(j1 + m1) * f(j2 - m2) * f(j2 + m2))
    kmin = max(0, j2 - j - m1, j1 - j + m2)
    kmax = min(j1 + j2 - j, j1 - m1, j2 + m2)
    s = 0.0
    for k in range(kmin, kmax + 1):
        s += (-1) ** k / (f(k) * f(j1 + j2 - j - k) * f(j1 - m1 - k)
                          * f(j2 + m2 - k) * f(j - j2 + m1 + k) * f(j - j1 - m2 + k))
    return pref * s


def _cg_matrix(l1, l2, l):
    M = np.zeros((2 * l1 + 1, 2 * l2 + 1, 2 * l + 1), dtype=np.float64)
    for i, m1 in enumerate(range(-l1, l1 + 1)):
        for j, m2 in enumerate(range(-l2, l2 + 1)):
            m = m1 + m2
            if -l <= m <= l:
                M[i, j, m + l] = _cg_coeff(l1, m1, l2, m2, l, m)
    return M


PAIRS = {l: [(l1, l2) for l1 in range(MAXL + 1) for l2 in range(MAXL + 1)
             if abs(l1 - l2) <= l <= l1 + l2] for l in range(MAXL + 1)}


def build_plan():
    unique_pairs, wmerge = {}, {}
    for l in range(MAXL + 1):
        ups, wm = [], []
        for pi, (l1, l2) in enumerate(PAIRS[l]):
            if l1 > l2:
                continue
            sign = (-1) ** (l1 + l2 - l)
            if l1 == l2 and sign == -1:
                continue
            merge = [(pi, 1.0)]
            if l1 != l2 and (l2, l1) in PAIRS[l]:
                merge.append((PAIRS[l].index((l2, l1)), float(sign)))
            ups.append((l1, l2))
            wm.append(merge)
        unique_pairs[l] = ups
        wmerge[l] = wm

    entries = {}   # ((l1,l2), i, j) -> [(l, u_idx, m, coeff)]
    for l in range(MAXL + 1):
        for u_idx, (l1, l2) in enumerate(unique_pairs[l]):
            M = _cg_matrix(l1, l2, l)
            for i in range(2 * l1 + 1):
                for j in range(2 * l2 + 1):
                    if l1 == l2 and j < i:
                        continue
                    for mm in range(2 * l + 1):
                        c = M[i, j, mm] + (M[j, i, mm] if (l1 == l2 and j != i) else 0.0)
                        if abs(c) < 1e-12:
                            continue
                        entries.setdefault(((l1, l2), i, j), []).append((l, u_idx, mm, float(c)))
    return unique_pairs, wmerge, entries


UNIQUE_PAIRS, WMERGE, ENTRIES = build_plan()
GROUP_ORDER = [(1, 1), (2, 2), (1, 2), (0, 2), (0, 0), (0, 1)]
L_READY = {2: 3, 0: 4, 1: 5}
N_U = {l: len(UNIQUE_PAIRS[l]) for l in range(MAXL + 1)}
KT = {l: 2 * N_U[l] * NCHT for l in range(MAXL + 1)}   # k'-tiles per l


def _plan_products():
    prods = {}
    for g in GROUP_ORDER:
        used = sorted((i, j) for (gg, i, j) in ENTRIES if gg == g)
        by_i = {}
        for i, j in used:
            by_i.setdefault(i, []).append(j)
        runs = []
        for i, js in sorted(by_i.items()):
            js = sorted(js)
            lo = prev = js[0]
            for j in js[1:]:
                if j == prev + 1:
                    prev = j
                else:
                    runs.append((i, lo, prev))
                    lo = prev = j
            runs.append((i, lo, prev))
        prods[g] = runs
    return prods


PROD_RUNS = _plan_products()


# ------------------------------------------------------------- host prep --


def _host_pack_inputs(x0, x1, x2, w0, w1, w2):
    xs = {0: np.asarray(x0), 1: np.asarray(x1), 2: np.asarray(x2)}
    ws = {0: np.asarray(w0), 1: np.asarray(w1), 2: np.asarray(w2)}
    xdt = np.dtype("float32")
    if PROD_DT16 == "fp16":
        xdt = np.dtype("float16")
    elif PROD_DT16 == "bf16":
        import ml_dtypes
        xdt = np.dtype(ml_dtypes.bfloat16)

    wtiles = {}
    for l in range(MAXL + 1):
        wc = ws[l][..., 0].astype(np.float32) + 1j * ws[l][..., 1].astype(np.float32)
        pieces = []
        for merge in WMERGE[l]:
            acc = np.zeros((CH, CH), dtype=np.complex64)
            for pi, sgn in merge:
                acc += sgn * wc[pi * CH:(pi + 1) * CH, :]
            pieces.append(acc)
        wt = np.concatenate(pieces, axis=0)          # [n_u*CH, CH] complex
        re, im = wt.real.astype(np.float32), wt.imag.astype(np.float32)
        top = np.concatenate([re, im], axis=1)       # plane 0 rows
        bot = np.concatenate([-im, re], axis=1)      # plane 1 rows
        Wp = np.concatenate([top, bot], axis=0)      # [2*n_u*CH, 512]
        Wp = Wp.reshape(KT[l], 128, 512)
        if PROD_DT16 == "fp16":
            Wp = Wp.astype(np.float16)
        wtiles[l] = np.ascontiguousarray(Wp)

    in_maps = []
    for c in range(N_CORES):
        m = {}
        for l in range(MAXL + 1):
            xl = xs[l][c * B_LOC:(c + 1) * B_LOC]     # [256, 2l+1, 256, 2]
            # -> [chunk, ri, chA(128 partitions), m, chB, bc]
            xt = xl.transpose(2, 3, 1, 0).reshape(2, 128, 2, 2 * l + 1, 2, BC)
            xt = xt.transpose(4, 2, 1, 3, 0, 5)
            m[f"x{l}"] = np.ascontiguousarray(xt.astype(np.float32)).astype(xdt)
            m[f"w{l}"] = wtiles[l]
        in_maps.append(m)
    return in_maps


# ---------------------------------------------------------- device kernel --


def _build_bass():
    import concourse.bass as bass
    import concourse.bacc as bacc
    import concourse.tile as tile
    from concourse import mybir

    f32 = mybir.dt.float32
    f32r = mybir.dt.float32r
    pdt = {None: f32, "fp16": mybir.dt.float16, "bf16": mybir.dt.bfloat16}[PROD_DT16]
    wdt = mybir.dt.float16 if PROD_DT16 == "fp16" else f32r
    mult = mybir.AluOpType.mult
    add = mybir.AluOpType.add
    sub = mybir.AluOpType.subtract

    nc = bacc.Bacc("TRN2", target_bir_lowering=False, debug=False)

    xd, wd, od = {}, {}, {}
    for l in range(MAXL + 1):
        nm = 2 * l + 1
        xd[l] = nc.dram_tensor(f"x{l}", [2, 2, 128, nm * 2 * BC], pdt, kind="ExternalInput").ap()
        wd[l] = nc.dram_tensor(f"w{l}", [KT[l], 128, 512], wdt, kind="ExternalInput").ap()
        od[l] = nc.dram_tensor(f"out{l}", [B_LOC, nm, 512], f32, kind="ExternalOutput").ap()

    n_per_block = {}
    for tgts in ENTRIES.values():
        for (l, u, mm, cf) in tgts:
            n_per_block[(l, u, mm)] = n_per_block.get((l, u, mm), 0) + 1

    with tile.TileContext(nc) as tc:
        ctx = ExitStack()
        with ctx:
            xpool = ctx.enter_context(tc.tile_pool(name="xp", bufs=2))
            stpool = ctx.enter_context(tc.tile_pool(name="st", bufs=1))
            kpool = ctx.enter_context(tc.tile_pool(name="kp", bufs=2))
            ppool = ctx.enter_context(tc.tile_pool(name="pp", bufs=2))
            bpool = ctx.enter_context(tc.tile_pool(name="bp", bufs=2))
            wpool = ctx.enter_context(tc.tile_pool(name="wp", bufs=1))
            opool = ctx.enter_context(tc.tile_pool(name="op", bufs=3))
            pspool = ctx.enter_context(tc.tile_pool(name="ps", bufs=6, space="PSUM"))

            wt = {}

            blocks = {}
            remaining = {}
            CB = 2 * BC   # (chB, bc) super-index: 128 high channels x 128 batch

            def emit_macs(g, i, j, P):
                """P: [128,512] (re256|im256, inner (chB,bc)) slice for (g,i,j)."""
                for (l, u, mm, cf) in ENTRIES[(g, i, j)]:
                    bkc = (l, u, mm)
                    if bkc not in blocks or remaining[bkc] == n_per_block[bkc]:
                        if bkc not in blocks:
                            blocks[bkc] = bpool.tile([128, 512], wdt, tag=f"b{l}_{u}_{mm}", name=f"b{l}_{u}_{mm}")
                            remaining[bkc] = n_per_block[bkc]
                        bt = blocks[bkc]
                        nc.vector.tensor_scalar(out=bt[:], in0=P, scalar1=cf, scalar2=None, op0=mult)
                    else:
                        bt = blocks[bkc]
                        nc.vector.scalar_tensor_tensor(out=bt[:], in0=P, scalar=cf, in1=bt[:], op0=mult, op1=add)
                    remaining[bkc] -= 1
                    if remaining[bkc] == 0:
                        remaining[bkc] = n_per_block[bkc]  # reset for next chunk

            def emit_matmuls(l, chunk):
                nm = 2 * l + 1
                for mm in range(nm):
                    ps = pspool.tile([128, 512], f32, tag="ps", name="ps")
                    T = KT[l]
                    for t in range(T):
                        plane = t // (N_U[l] * 2)
                        rem = t % (N_U[l] * 2)
                        u = rem // 2
                        chb = rem % 2
                        bt = blocks[(l, u, mm)]
                        off = plane * 256 + chb * 128
                        nc.tensor.matmul(ps[:], bt[:, off:off + 128], wt[(l, t)][:],
                                         start=(t == 0), stop=(t == T - 1))
                    ot = opool.tile([128, 512], f32, tag="ot", name="ot")
                    nc.scalar.copy(ot[:], ps[:])
                    nc.sync.dma_start(od[l][chunk * BC:(chunk + 1) * BC, mm], ot[:])

            # ---- per chunk: load x chunk, s/tm, products -> MACs -> matmuls ----
            for chunk in range(2):
                xt, st, tmt = {}, {}, {}
                for l in range(MAXL + 1):
                    nm = 2 * l + 1
                    for ri in range(2):
                        t = xpool.tile([128, nm * CB], pdt, tag=f"x{l}_{ri}", name=f"x{l}_{ri}")
                        nc.sync.dma_start(t[:], xd[l][chunk, ri])
                        xt[(l, ri)] = t
                    s = stpool.tile([128, nm * CB], pdt, tag=f"s{l}", name=f"s{l}")
                    nc.vector.tensor_tensor(s[:], xt[(l, 0)][:], xt[(l, 1)][:], add)
                    st[l] = s
                    tm = stpool.tile([128, nm * CB], pdt, tag=f"tm{l}", name=f"tm{l}")
                    nc.vector.tensor_tensor(tm[:], xt[(l, 1)][:], xt[(l, 0)][:], sub)
                    tmt[l] = tm

                if chunk == 0:
                    # preload W tiles now (after x DMAs) so DVE starts early
                    for l in (2, 0, 1):
                        for t in range(KT[l]):
                            w = wpool.tile([128, 512], wdt, tag=f"w{l}_{t}", name=f"w{l}_{t}")
                            nc.sync.dma_start(w[:], wd[l][t])
                            wt[(l, t)] = w

                for gi, g in enumerate(GROUP_ORDER):
                    l1, l2 = g
                    for (i, vlo, vhi) in PROD_RUNS[g]:
                        nv = vhi - vlo + 1

                        s1 = st[l1][:, i * CB:(i + 1) * CB]
                        xre1 = xt[(l1, 0)][:, i * CB:(i + 1) * CB]
                        xim1 = xt[(l1, 1)][:, i * CB:(i + 1) * CB]

                        P = ppool.tile([128, nv * 512], pdt, tag="pg", name="pg")
                        Pv = P[:].rearrange("p (v two cb) -> p v two cb", v=nv, two=2)
                        Pre = Pv[:, :, 0, :]
                        Pim = Pv[:, :, 1, :]

                        def bc(ap):
                            return ap.unsqueeze(1).broadcast_to((128, nv, CB))

                        def vsl(tile_):
                            # 3D view [128, nv, CB] over the v-run
                            nmv = tile_.shape[1] // CB
                            t3 = tile_[:].rearrange("p (v cb) -> p v cb", v=nmv)
                            return t3[:, vlo:vhi + 1, :]

                        xre2v = vsl(xt[(l2, 0)])
                        tm2v = vsl(tmt[l2])
                        s2v = vsl(st[l2])

                        # pass1: P.re = K1 = xre_v * s_u
                        nc.vector.tensor_tensor(Pre, xre2v, bc(s1), mult)
                        # pass2: P.im = K2 = tm_v * xre_u
                        nc.vector.tensor_tensor(Pim, tm2v, bc(xre1), mult)
                        # pass3: P.im += P.re  (k1+k2)
                        nc.vector.tensor_tensor(Pim, Pim, Pre, add)
                        # pass4: K3 = s_v * xim_u
                        K3 = kpool.tile([128, nv * CB], pdt, tag="k3", name="k3")
                        K3v = K3[:].rearrange("p (v cb) -> p v cb", v=nv)
                        nc.vector.tensor_tensor(K3v, s2v, bc(xim1), mult)
                        # pass5: P.re -= K3
                        nc.vector.tensor_tensor(Pre, Pre, K3v, sub)

                        for j in range(vlo, vhi + 1):
                            if ((l1, l2), i, j) not in ENTRIES:
                                continue
                            vloc = j - vlo
                            emit_macs(g, i, j, P[:, vloc * 512:(vloc + 1) * 512])

                    for l in range(MAXL + 1):
                        if L_READY[l] == gi:
                            emit_matmuls(l, chunk)

    nc.compile()
    return nc


_NC_CACHE = {}


def _get_nc():
    key = ("nc", PROD_DT16)
    if key not in _NC_CACHE:
        _NC_CACHE[key] = _build_bass()
    return _NC_CACHE[key]


def kernel(x0, x1, x2, w0, w1, w2):
    from concourse import bass_utils

    in_maps = _host_pack_inputs(x0, x1, x2, w0, w1, w2)
    nc = _get_nc()
    try:
        res = bass_utils.run_bass_kernel_spmd(nc, in_maps, core_ids=list(range(N_CORES)))
    except Exception:
        # transient device faults (NRT unrecoverable) clear on retry
        import time
        time.sleep(2.0)
        res = bass_utils.run_bass_kernel_spmd(nc, in_maps, core_ids=list(range(N_CORES)))
    outs = []
    for l in range(MAXL + 1):
        parts = [res.results[c][f"out{l}"].reshape(B_LOC, 2 * l + 1, 2, CH)
                 .transpose(0, 1, 3, 2)
                 for c in range(N_CORES)]
        outs.append(np.ascontiguousarray(np.concatenate(parts, axis=0)))
    return tuple(outs)


# revision 46
# speedup vs baseline: 1.0171x; 1.0042x over previous
"""Trainium2 Bass kernel for nn_CGNonLinearity3D (MAXL=2, CH=256, B=2048).

Algorithm (per core, batch-sharded x8):
  stage 1 (DVE): complex "diagonal CG" products via Gauss 3-mult, weighted
    accumulation into symmetry-reduced cat blocks (zero blocks dropped,
    (l2,l1) duplicates folded into host-merged weights).
  stage 2 (PE): per-(l,m) complex matmul as real f32r matmuls accumulating
    in PSUM; batch rows on PSUM partitions, merged weights as moving operand.
Output rows are [re(256) | im(256)]; host un-interleaves.
"""
import numpy as np
from math import factorial, sqrt
from contextlib import ExitStack

MAXL = 2
CH = 256
B = 2048
N_CORES = 8
B_LOC = B // N_CORES          # 256
BC = 128                      # batch chunk (2 chunks per core)
NCHT = CH // 128              # 2 channel partition-tiles

PROD_DT16 = "fp16"          # None (fp32) | "fp16" | "bf16" stage-1 product dtype

# ---------------------------------------------------------------- CG plan --


def _cg_coeff(j1, m1, j2, m2, j, m):
    if m1 + m2 != m or not (abs(j1 - j2) <= j <= j1 + j2):
        return 0.0
    f = factorial
    pref = sqrt((2 * j + 1) * f(j + j1 - j2) * f(j - j1 + j2) * f(j1 + j2 - j) / f(j1 + j2 + j + 1))
    pref *= sqrt(f(j + m) * f(j - m) * f(j1 - m1) * f(j1 + m1) * f(j2 - m2) * f(j2 + m2))
    kmin = max(0, j2 - j - m1, j1 - j + m2)
    kmax = min(j1 + j2 - j, j1 - m1, j2 + m2)
    s = 0.0
    for k in range(kmin, kmax + 1):
        s += (-1) ** k / (f(k) * f(j1 + j2 - j - k) * f(j1 - m1 - k)
                          * f(j2 + m2 - k) * f(j - j2 + m1 + k) * f(j - j1 - m2 + k))
    return pref * s


def _cg_matrix(l1, l2, l):
    M = np.zeros((2 * l1 + 1, 2 * l2 + 1, 2 * l + 1), dtype=np.float64)
    for i, m1 in enumerate(range(-l1, l1 + 1)):
        for j, m2 in enumerate(range(-l2, l2 + 1)):
            m = m1 + m2
            if -l <= m <= l:
                M[i, j, m + l] = _cg_coeff(l1, m1, l2, m2, l, m)
    return M


PAIRS = {l: [(l1, l2) for l1 in range(MAXL + 1) for l2 in range(MAXL + 1)
             if abs(l1 - l2) <= l <= l1 + l2] for l in range(MAXL + 1)}


def build_plan():
    unique_pairs, wmerge = {}, {}
    for l in range(MAXL + 1):
        ups, wm = [], []
        for pi, (l1, l2) in enumerate(PAIRS[l]):
            if l1 > l2:
                continue
            sign = (-1) ** (l1 + l2 - l)
            if l1 == l2 and sign == -1:
                continue
            merge = [(pi, 1.0)]
            if l1 != l2 and (l2, l1) in PAIRS[l]:
                merge.append((PAIRS[l].index((l2, l1)), float(sign)))
            ups.append((l1, l2))
            wm.append(merge)
        unique_pairs[l] = ups
        wmerge[l] = wm

    entries = {}   # ((l1,l2), i, j) -> [(l, u_idx, m, coeff)]
    for l in range(MAXL + 1):
        for u_idx, (l1, l2) in enumerate(unique_pairs[l]):
            M = _cg_matrix(l1, l2, l)
            for i in range(2 * l1 + 1):
                for j in range(2 * l2 + 1):
                    if l1 == l2 and j < i:
                        continue
                    for mm in range(2 * l + 1):
                        c = M[i, j, mm] + (M[j, i, mm] if (l1 == l2 and j != i) else 0.0)
                        if abs(c) < 1e-12:
                            continue
                        entries.setdefault(((l1, l2), i, j), []).append((l, u_idx, mm, float(c)))
    return unique_pairs, wmerge, entries


UNIQUE_PAIRS, WMERGE, ENTRIES = build_plan()
GROUP_ORDER = [(1, 1), (2, 2), (1, 2), (0, 2), (0, 0), (0, 1)]
L_READY = {2: 3, 0: 4, 1: 5}
N_U = {l: len(UNIQUE_PAIRS[l]) for l in range(MAXL + 1)}
KT = {l: 2 * N_U[l] * NCHT for l in range(MAXL + 1)}   # k'-tiles per l


def _plan_products():
    prods = {}
    for g in GROUP_ORDER:
        used = sorted((i, j) for (gg, i, j) in ENTRIES if gg == g)
        by_i = {}
        for i, j in used:
            by_i.setdefault(i, []).append(j)
        runs = []
        for i, js in sorted(by_i.items()):
            js = sorted(js)
            lo = prev = js[0]
            for j in js[1:]:
                if j == prev + 1:
                    prev = j
                else:
                    runs.append((i, lo, prev))
                    lo = prev = j
            runs.append((i, lo, prev))
        prods[g] = runs
    return prods


PROD_RUNS = _plan_products()


# ------------------------------------------------------------- host prep --


def _host_pack_inputs(x0, x1, x2, w0, w1, w2):
    xs = {0: np.asarray(x0), 1: np.asarray(x1), 2: np.asarray(x2)}
    ws = {0: np.asarray(w0), 1: np.asarray(w1), 2: np.asarray(w2)}
    xdt = np.dtype("float32")
    if PROD_DT16 == "fp16":
        xdt = np.dtype("float16")
    elif PROD_DT16 == "bf16":
        import ml_dtypes
        xdt = np.dtype(ml_dtypes.bfloat16)

    wtiles = {}
    for l in range(MAXL + 1):
        wc = ws[l][..., 0].astype(np.float32) + 1j * ws[l][..., 1].astype(np.float32)
        pieces = []
        for merge in WMERGE[l]:
            acc = np.zeros((CH, CH), dtype=np.complex64)
            for pi, sgn in merge:
                acc += sgn * wc[pi * CH:(pi + 1) * CH, :]
            pieces.append(acc)
        wt = np.concatenate(pieces, axis=0)          # [n_u*CH, CH] complex
        re, im = wt.real.astype(np.float32), wt.imag.astype(np.float32)
        top = np.concatenate([re, im], axis=1)       # plane 0 rows
        bot = np.concatenate([-im, re], axis=1)      # plane 1 rows
        Wp = np.concatenate([top, bot], axis=0)      # [2*n_u*CH, 512]
        Wp = Wp.reshape(KT[l], 128, 512)
        if PROD_DT16 == "fp16":
            Wp = Wp.astype(np.float16)
        wtiles[l] = np.ascontiguousarray(Wp)

    in_maps = []
    for c in range(N_CORES):
        m = {}
        for l in range(MAXL + 1):
            xl = xs[l][c * B_LOC:(c + 1) * B_LOC]     # [256, 2l+1, 256, 2]
            # -> [chunk, ri, chA(128 partitions), m, chB, bc]
            xt = xl.transpose(2, 3, 1, 0).reshape(2, 128, 2, 2 * l + 1, 2, BC)
            xt = xt.transpose(4, 2, 1, 3, 0, 5)
            m[f"x{l}"] = np.ascontiguousarray(xt.astype(np.float32)).astype(xdt)
            m[f"w{l}"] = wtiles[l]
        in_maps.append(m)
    return in_maps


# ---------------------------------------------------------- device kernel --


def _build_bass():
    import concourse.bass as bass
    import concourse.bacc as bacc
    import concourse.tile as tile
    from concourse import mybir

    f32 = mybir.dt.float32
    f32r = mybir.dt.float32r
    pdt = {None: f32, "fp16": mybir.dt.float16, "bf16": mybir.dt.bfloat16}[PROD_DT16]
    wdt = mybir.dt.float16 if PROD_DT16 == "fp16" else f32r
    mult = mybir.AluOpType.mult
    add = mybir.AluOpType.add
    sub = mybir.AluOpType.subtract

    nc = bacc.Bacc("TRN2", target_bir_lowering=False, debug=False)

    xd, wd, od = {}, {}, {}
    for l in range(MAXL + 1):
        nm = 2 * l + 1
        xd[l] = nc.dram_tensor(f"x{l}", [2, 2, 128, nm * 2 * BC], pdt, kind="ExternalInput").ap()
        wd[l] = nc.dram_tensor(f"w{l}", [KT[l], 128, 512], wdt, kind="ExternalInput").ap()
        od[l] = nc.dram_tensor(f"out{l}", [B_LOC, nm, 512], f32, kind="ExternalOutput").ap()

    n_per_block = {}
    for tgts in ENTRIES.values():
        for (l, u, mm, cf) in tgts:
            n_per_block[(l, u, mm)] = n_per_block.get((l, u, mm), 0) + 1

    with tile.TileContext(nc) as tc:
        ctx = ExitStack()
        with ctx:
            xpool = ctx.enter_context(tc.tile_pool(name="xp", bufs=2))
            stpool = ctx.enter_context(tc.tile_pool(name="st", bufs=1))
            kpool = ctx.enter_context(tc.tile_pool(name="kp", bufs=1))
            ppool = ctx.enter_context(tc.tile_pool(name="pp", bufs=1))
            bpool = ctx.enter_context(tc.tile_pool(name="bp", bufs=2))
            wpool = ctx.enter_context(tc.tile_pool(name="wp", bufs=1))
            opool = ctx.enter_context(tc.tile_pool(name="op", bufs=3))
            pspool = ctx.enter_context(tc.tile_pool(name="ps", bufs=6, space="PSUM"))

            wt = {}

            blocks = {}
            remaining = {}
            CB = 2 * BC   # (chB, bc) super-index: 128 high channels x 128 batch

            def emit_macs(g, i, j, P):
                """P: [128,512] (re256|im256, inner (chB,bc)) slice for (g,i,j)."""
                for (l, u, mm, cf) in ENTRIES[(g, i, j)]:
                    bkc = (l, u, mm)
                    if bkc not in blocks or remaining[bkc] == n_per_block[bkc]:
                        if bkc not in blocks:
                            blocks[bkc] = bpool.tile([128, 512], wdt, tag=f"b{l}_{u}_{mm}", name=f"b{l}_{u}_{mm}")
                            remaining[bkc] = n_per_block[bkc]
                        bt = blocks[bkc]
                        nc.vector.tensor_scalar(out=bt[:], in0=P, scalar1=cf, scalar2=None, op0=mult)
                    else:
                        bt = blocks[bkc]
                        nc.vector.scalar_tensor_tensor(out=bt[:], in0=P, scalar=cf, in1=bt[:], op0=mult, op1=add)
                    remaining[bkc] -= 1
                    if remaining[bkc] == 0:
                        remaining[bkc] = n_per_block[bkc]  # reset for next chunk

            def emit_matmuls(l, chunk):
                nm = 2 * l + 1
                for mm in range(nm):
                    ps = pspool.tile([128, 512], f32, tag="ps", name="ps")
                    T = KT[l]
                    for t in range(T):
                        plane = t // (N_U[l] * 2)
                        rem = t % (N_U[l] * 2)
                        u = rem // 2
                        chb = rem % 2
                        bt = blocks[(l, u, mm)]
                        off = plane * 256 + chb * 128
                        nc.tensor.matmul(ps[:], bt[:, off:off + 128], wt[(l, t)][:],
                                         start=(t == 0), stop=(t == T - 1))
                    ot = opool.tile([128, 512], f32, tag="ot", name="ot")
                    nc.scalar.copy(ot[:], ps[:])
                    nc.sync.dma_start(od[l][chunk * BC:(chunk + 1) * BC, mm], ot[:])

            # ---- per chunk: load x chunk, s/tm, products -> MACs -> matmuls ----
            for chunk in range(2):
                xt, st, tmt = {}, {}, {}
                for l in range(MAXL + 1):
                    nm = 2 * l + 1
                    for ri in range(2):
                        t = xpool.tile([128, nm * CB], pdt, tag=f"x{l}_{ri}", name=f"x{l}_{ri}")
                        nc.sync.dma_start(t[:], xd[l][chunk, ri])
                        xt[(l, ri)] = t
                    s = stpool.tile([128, nm * CB], pdt, tag=f"s{l}", name=f"s{l}")
                    nc.vector.tensor_tensor(s[:], xt[(l, 0)][:], xt[(l, 1)][:], add)
                    st[l] = s
                    tm = stpool.tile([128, nm * CB], pdt, tag=f"tm{l}", name=f"tm{l}")
                    nc.vector.tensor_tensor(tm[:], xt[(l, 1)][:], xt[(l, 0)][:], sub)
                    tmt[l] = tm

                if chunk == 0:
                    # preload W tiles now (after x DMAs) so DVE starts early
                    for l in (2, 0, 1):
                        for t in range(KT[l]):
                            w = wpool.tile([128, 512], wdt, tag=f"w{l}_{t}", name=f"w{l}_{t}")
                            nc.sync.dma_start(w[:], wd[l][t])
                            wt[(l, t)] = w

                for gi, g in enumerate(GROUP_ORDER):
                    l1, l2 = g
                    runs = PROD_RUNS[g]
                    nslots = sum(vhi - vlo + 1 for (i, vlo, vhi) in runs)
                    P = ppool.tile([128, nslots * 512], pdt, tag="pg", name="pg")
                    K3 = kpool.tile([128, nslots * 256], pdt, tag="k3", name="k3")

                    # phase 1: per-run multiply passes into group-wide slices
                    base = 0
                    run_base = []
                    for (i, vlo, vhi) in runs:
                        nv = vhi - vlo + 1
                        run_base.append(base)

                        s1 = st[l1][:, i * CB:(i + 1) * CB]
                        xre1 = xt[(l1, 0)][:, i * CB:(i + 1) * CB]
                        xim1 = xt[(l1, 1)][:, i * CB:(i + 1) * CB]

                        Pv = P[:, base * 512:(base + nv) * 512].rearrange(
                            "p (v two cb) -> p v two cb", v=nv, two=2)
                        Pre = Pv[:, :, 0, :]
                        Pim = Pv[:, :, 1, :]
                        K3v = K3[:, base * 256:(base + nv) * 256].rearrange(
                            "p (v cb) -> p v cb", v=nv)

                        def bc(ap):
                            return ap.unsqueeze(1).broadcast_to((128, nv, CB))

                        def vsl(tile_):
                            nmv = tile_.shape[1] // CB
                            t3 = tile_[:].rearrange("p (v cb) -> p v cb", v=nmv)
                            return t3[:, vlo:vhi + 1, :]

                        nc.vector.tensor_tensor(Pre, vsl(xt[(l2, 0)]), bc(s1), mult)
                        nc.vector.tensor_tensor(Pim, vsl(tmt[l2]), bc(xre1), mult)
                        nc.vector.tensor_tensor(K3v, vsl(st[l2]), bc(xim1), mult)
                        base += nv

                    # phase 2: group-wide add/sub (k1+k2 and k1-k3)
                    PA = P[:].rearrange("p (v two cb) -> p v two cb", v=nslots, two=2)
                    PreA = PA[:, :, 0, :]
                    PimA = PA[:, :, 1, :]
                    K3A = K3[:].rearrange("p (v cb) -> p v cb", v=nslots)
                    nc.vector.tensor_tensor(PimA, PimA, PreA, add)
                    nc.vector.tensor_tensor(PreA, PreA, K3A, sub)

                    # phase 3: MACs
                    for ridx, (i, vlo, vhi) in enumerate(runs):
                        for j in range(vlo, vhi + 1):
                            if ((l1, l2), i, j) not in ENTRIES:
                                continue
                            slot = run_base[ridx] + (j - vlo)
                            emit_macs(g, i, j, P[:, slot * 512:(slot + 1) * 512])

                    for l in range(MAXL + 1):
                        if L_READY[l] == gi:
                            emit_matmuls(l, chunk)

    nc.compile()
    return nc


_NC_CACHE = {}


def _get_nc():
    key = ("nc", PROD_DT16)
    if key not in _NC_CACHE:
        _NC_CACHE[key] = _build_bass()
    return _NC_CACHE[key]


def kernel(x0, x1, x2, w0, w1, w2):
    from concourse import bass_utils

    in_maps = _host_pack_inputs(x0, x1, x2, w0, w1, w2)
    nc = _get_nc()
    try:
        res = bass_utils.run_bass_kernel_spmd(nc, in_maps, core_ids=list(range(N_CORES)))
    except Exception:
        # transient device faults (NRT unrecoverable) clear on retry
        import time
        time.sleep(2.0)
        res = bass_utils.run_bass_kernel_spmd(nc, in_maps, core_ids=list(range(N_CORES)))
    outs = []
    for l in range(MAXL + 1):
        parts = [res.results[c][f"out{l}"].reshape(B_LOC, 2 * l + 1, 2, CH)
                 .transpose(0, 1, 3, 2)
                 for c in range(N_CORES)]
        outs.append(np.ascontiguousarray(np.concatenate(parts, axis=0)))
    return tuple(outs)
